# revision 1
# baseline (speedup 1.0000x reference)
"""NonLocalBlock2D (embedded-gaussian non-local attention) on 8 TRN2 NeuronCores.

Sharding: data-parallel over (batch, query-half). Core k handles sample b=k//2,
query rows h*3200:(h+1)*3200 with h=k%2. Attention keys/values are the full
6400 positions of that sample; the small 1x1-conv / BN params are replicated.

Per-core program (SPMD, one Bass module for all 8 cores):
  theta = Wth @ x_q + bth          [32,3200]  (stored 4x-replicated -> [128,3200])
  phi   = Wph @ x   + bph          [32,6400]  (4x-replicated -> [128,6400])
  gT    = x.T @ WgT + bg, chunked  [128,33] x 50  (col 32 = ones, for denominators)
  for each query block (512):
    for each key-chunk group (3 chunks of 128 keys, row-packed matmuls):
      fT = phi_chunk.T @ theta_blk -> PSUM [128,3x512]
      e  = exp(fT)                 -> SBUF  (ScalarE, the bottleneck engine)
      y  += gT_chunk.T @ e         -> PSUM [33,512] (row 32 accumulates denom)
    r = 1/denom; out = (WoT.T @ y) * r + x_residual   (BN folded into Wo/bias)

Host folds BN into the output conv, rotates x per-core so the query block is
always columns 0:3200 (softmax is invariant to key permutation), and stitches
the 8 [64,3200] results back into [4,64,80,80].
"""

import numpy as np

import concourse.bass as bass
import concourse.tile as tile
from concourse import bacc
from concourse import mybir
from concourse.bass import ts
from concourse.bass_utils import run_bass_kernel_spmd

B, C, HH, WW = 4, 64, 80, 80
N = HH * WW            # 6400 key positions per sample
NQ = N // 2            # 3200 query rows per core
INTER = 32
NCORES = 8

MC = 128               # keys per PE chunk
NMC = N // MC          # 50 chunks
PACK = 3               # chunks per packed f-matmul group (3 PSUM banks)
NB = 512               # query block size

F32 = mybir.dt.float32
F32R = mybir.dt.float32r
EXP = mybir.ActivationFunctionType.Exp
ADD = mybir.AluOpType.add
MULT = mybir.AluOpType.mult

BN_EPS = 1e-4

# r-broadcast strategy: 'dve' = stride-0 partition read on DVE,
# 'dma' = materialize via DMA partition-broadcast
RBC_MODE = 'dma'


def _blocks(total, size):
    off = 0
    while off < total:
        sz = min(size, total - off)
        yield off, sz
        off += sz


DEBUG = False


def _emit(tc, d, repeat=1):
    nc = tc.nc

    with tc.tile_pool(name="singles", bufs=1) as singles:
        wth = singles.tile([C, 128], F32, tag="wth")
        nc.sync.dma_start(wth[:], d["wth"][:])
        wph = singles.tile([C, 128], F32, tag="wph")
        nc.sync.dma_start(wph[:], d["wph"][:])
        wg = singles.tile([C, INTER], F32, tag="wg")
        nc.sync.dma_start(wg[:], d["wg"][:])
        wo = singles.tile([INTER, C], F32, tag="wo")
        nc.sync.dma_start(wo[:], d["wo"][:])
        bth = singles.tile([128, 1], F32, tag="bth")
        nc.sync.dma_start(bth[:], d["bth"][:])
        bph = singles.tile([128, 1], F32, tag="bph")
        nc.sync.dma_start(bph[:], d["bph"][:])
        bg = singles.tile([128, INTER], F32, tag="bg")
        nc.sync.dma_start(bg[:], d["bg"][0:1, :].partition_broadcast(128))
        ones64 = singles.tile([1, C], F32, tag="ones64")
        nc.vector.memset(ones64[:], 1.0)

        xfl = singles.tile([C, N], F32, tag="xf")
        for off, sz in _blocks(N, 3072):
            nc.sync.dma_start(xfl[:, off : off + sz], d["xf"][:, off : off + sz])
        xrl = singles.tile([C, NQ], F32, tag="xr")
        for off, sz in _blocks(NQ, 3072):
            nc.sync.dma_start(xrl[:, off : off + sz], d["xr"][:, off : off + sz])

        xfr = singles.tile([C, N], F32R, tag="xfr")
        nc.vector.tensor_copy(xfr[:], xfl[:])
        wthr = singles.tile([C, 128], F32R, tag="wthr")
        nc.vector.tensor_copy(wthr[:], wth[:])
        wphr = singles.tile([C, 128], F32R, tag="wphr")
        nc.vector.tensor_copy(wphr[:], wph[:])
        th = singles.tile([128, NQ], F32R, tag="th")

        ph = singles.tile([128, N], F32R, tag="ph")
        gt = singles.tile([128, NMC, INTER + 1], F32R, tag="gt")
        onescol = singles.tile([128, NMC], F32, tag="onescol")
        nc.vector.memset(onescol[:], 1.0)
        nc.vector.tensor_copy(gt[:, :, INTER : INTER + 1], onescol[:].rearrange("p (n o) -> p n o", o=1))

        # ---- input 1x1 convs ----
        for _rep in range(repeat):
            with tc.tile_pool(name="cpsum", bufs=4, space="PSUM") as cpsum:
                for off, sz in _blocks(NQ, NB):
                    pt = cpsum.tile([128, NB], F32, tag="cps")
                    nc.tensor.matmul(
                        pt[:, :sz],
                        lhsT=wthr[:],
                        rhs=xfr[:, off : off + sz],
                        start=True,
                        stop=True,
                    )
                    nc.vector.tensor_scalar_add(th[:, off : off + sz], pt[:, :sz], bth[:])
                for off, sz in _blocks(N, NB):
                    pp = cpsum.tile([128, NB], F32, tag="cps")
                    nc.tensor.matmul(
                        pp[:, :sz],
                        lhsT=wphr[:],
                        rhs=xfr[:, off : off + sz],
                        start=True,
                        stop=True,
                    )
                    nc.vector.tensor_scalar_add(ph[:, off : off + sz], pp[:, :sz], bph[:])
                for k in range(NMC):
                    pg = cpsum.tile([128, NB], F32, tag="cps")
                    nc.tensor.matmul(
                        pg[:, :INTER],
                        lhsT=xfl[:, ts(k, MC)],
                        rhs=wg[:],
                        start=True,
                        stop=True,
                    )
                    nc.vector.tensor_tensor(gt[:, k, :INTER], pg[:, :INTER], bg[:], op=ADD)

            if DEBUG:
                nc.sync.dma_start(d["d_th"][:], th[:].bitcast(F32))
                nc.sync.dma_start(d["d_ph"][:], ph[:].bitcast(F32))
                nc.sync.dma_start(d["d_gt"][:], gt[:].rearrange("p a b -> p (a b)").bitcast(F32))
                nc.sync.dma_start(d["d_bg"][:], bg[:])

            # ---- attention ----
            groups = []
            c0 = 0
            while c0 < NMC:
                gsz = min(PACK, NMC - c0)
                groups.append((c0, gsz))
                c0 += gsz

            att_blocks = [(0, 512), (512, 512), (1024, 512), (1536, 512), (2048, 512), (2560, 384), (2944, 256)]
            with tc.tile_pool(name="fpsum", bufs=2, space="PSUM") as fpsum, tc.tile_pool(
                name="ypsum", bufs=2, space="PSUM"
            ) as ypsum, tc.tile_pool(name="esb", bufs=3) as esb, tc.tile_pool(
                name="ep", bufs=2
            ) as ep:
                for n0, nb in att_blocks:
                    py = ypsum.tile([INTER + 1, NB], F32, tag="yz")
                    pending = [None]

                    def flush_y(py=py, nb=nb, pending=pending):
                        if pending[0] is None:
                            return
                        e, c0p, gszp = pending[0]
                        for j in range(gszp):
                            ch = c0p + j
                            nc.tensor.matmul(
                                py[:, :nb],
                                lhsT=gt[:, ch, :],
                                rhs=e[:, j, :nb],
                                start=(ch == 0),
                                stop=(ch == NMC - 1),
                            )
                        pending[0] = None

                    for c0g, gsz in groups:
                        pf = fpsum.tile([128, PACK, NB], F32, tag="f")
                        for j in range(gsz):
                            ch = c0g + j
                            bp = 32 * j
                            nc.tensor.matmul(
                                pf[:, j, :nb],
                                lhsT=ph[bp : bp + 32, ts(ch, MC)],
                                rhs=th[bp : bp + 32, n0 : n0 + nb],
                                start=True,
                                stop=True,
                            )
                        flush_y()
                        e = esb.tile([128, PACK, NB], F32R, tag="e")
                        nc.scalar.activation(e[:, :gsz, :nb], pf[:, :gsz, :nb], EXP)
                        if DEBUG and n0 == 0 and c0g == 0:
                            nc.sync.dma_start(d["d_e"][:], e[:].rearrange("p a b -> p (a b)").bitcast(F32))
                        pending[0] = (e, c0g, gsz)
                    flush_y()

                    # ---- block epilogue: normalize, output conv, residual ----
                    r = ep.tile([1, NB], F32, tag="r")
                    scr = ep.tile([1, NB], F32, tag="scr")
                    den = ep.tile([1, NB], F32, tag="den")
                    nc.vector.tensor_copy(den[:, :nb], py[INTER : INTER + 1, :nb])
                    nc.vector.reciprocal_approx_accurate(r[:, :nb], den[:, :nb], scr[:, :nb])
                    ysb = ep.tile([INTER, NB], F32, tag="ysb")
                    nc.vector.tensor_copy(ysb[:, :nb], py[:INTER, :nb])
                    if DEBUG and n0 == 0:
                        nc.sync.dma_start(d["d_ysb"][:], ysb[:, :nb])
                        nc.sync.dma_start(d["d_r"][:], r[:, :nb])
                    z = ypsum.tile([C, NB], F32, tag="yz")
                    nc.tensor.matmul(z[:, :nb], lhsT=wo[:], rhs=ysb[:, :nb], start=True, stop=True)
                    rbp = ypsum.tile([C, NB], F32, tag="yz")
                    nc.tensor.matmul(rbp[:, :nb], lhsT=ones64[:], rhs=r[:, :nb], start=True, stop=True)
                    rbc = ep.tile([C, NB], F32, tag="rbc")
                    nc.vector.tensor_copy(rbc[:, :nb], rbp[:, :nb])
                    if DEBUG and n0 == 0:
                        nc.sync.dma_start(d["d_rbc"][:], rbc[:, :nb])
                    t = ep.tile([C, NB], F32, tag="t")
                    nc.vector.tensor_tensor(t[:, :nb], z[:, :nb], rbc[:, :nb], op=MULT)
                    o = ep.tile([C, NB], F32, tag="o")
                    nc.vector.tensor_tensor(o[:, :nb], t[:, :nb], xrl[:, n0 : n0 + nb], op=ADD)
                    nc.sync.dma_start(d["out"][:, n0 : n0 + nb], o[:, :nb])


def build(repeat=1):
    nc = bacc.Bacc("TRN2", target_bir_lowering=False, debug=False)
    d = {}
    d["xf"] = nc.dram_tensor("xf", [C, N], F32, kind="ExternalInput").ap()
    d["xr"] = nc.dram_tensor("xr", [C, NQ], F32, kind="ExternalInput").ap()
    d["wth"] = nc.dram_tensor("wth", [C, 128], F32, kind="ExternalInput").ap()
    d["wph"] = nc.dram_tensor("wph", [C, 128], F32, kind="ExternalInput").ap()
    d["wg"] = nc.dram_tensor("wg", [C, INTER], F32, kind="ExternalInput").ap()
    d["wo"] = nc.dram_tensor("wo", [INTER, C], F32, kind="ExternalInput").ap()
    d["bth"] = nc.dram_tensor("bth", [128, 1], F32, kind="ExternalInput").ap()
    d["bph"] = nc.dram_tensor("bph", [128, 1], F32, kind="ExternalInput").ap()
    d["bg"] = nc.dram_tensor("bg", [1, INTER], F32, kind="ExternalInput").ap()
    d["out"] = nc.dram_tensor("out", [C, NQ], F32, kind="ExternalOutput").ap()
    if DEBUG:
        d["d_th"] = nc.dram_tensor("d_th", [128, NQ], F32, kind="ExternalOutput").ap()
        d["d_ph"] = nc.dram_tensor("d_ph", [128, N], F32, kind="ExternalOutput").ap()
        d["d_gt"] = nc.dram_tensor("d_gt", [128, NMC * (INTER + 1)], F32, kind="ExternalOutput").ap()
        d["d_bg"] = nc.dram_tensor("d_bg", [128, INTER], F32, kind="ExternalOutput").ap()
        d["d_e"] = nc.dram_tensor("d_e", [128, PACK * NB], F32, kind="ExternalOutput").ap()
        d["d_ysb"] = nc.dram_tensor("d_ysb", [INTER, NB], F32, kind="ExternalOutput").ap()
        d["d_r"] = nc.dram_tensor("d_r", [1, NB], F32, kind="ExternalOutput").ap()
        d["d_rbc"] = nc.dram_tensor("d_rbc", [C, NB], F32, kind="ExternalOutput").ap()
    with tile.TileContext(nc) as tc:
        _emit(tc, d, repeat=repeat)
    nc.compile()
    return nc


def make_in_maps(x, w_theta, b_theta, w_phi, b_phi, w_g, b_g,
                 w_out, b_out, bn_gamma, bn_beta, bn_mean, bn_var):
    x = np.ascontiguousarray(np.asarray(x, dtype=np.float32))
    w_theta = np.asarray(w_theta, np.float32)
    b_theta = np.asarray(b_theta, np.float32)
    w_phi = np.asarray(w_phi, np.float32)
    b_phi = np.asarray(b_phi, np.float32)
    w_g = np.asarray(w_g, np.float32)
    b_g = np.asarray(b_g, np.float32)
    w_out = np.asarray(w_out, np.float32)
    b_out = np.asarray(b_out, np.float32)
    bn_gamma = np.asarray(bn_gamma, np.float32)
    bn_beta = np.asarray(bn_beta, np.float32)
    bn_mean = np.asarray(bn_mean, np.float32)
    bn_var = np.asarray(bn_var, np.float32)

    inv = bn_gamma / np.sqrt(bn_var + BN_EPS)
    wo_folded = w_out * inv[:, None]                       # [64,32]
    bo_folded = (b_out - bn_mean) * inv + bn_beta          # [64]

    wth4 = np.ascontiguousarray(np.tile(w_theta.T, (1, 4)))   # [64,128]
    wph4 = np.ascontiguousarray(np.tile(w_phi.T, (1, 4)))     # [64,128]
    wg_r = np.ascontiguousarray(w_g.T)                        # [64,32]
    wo_l = np.ascontiguousarray(wo_folded.T)                  # [32,64]
    bth4 = np.ascontiguousarray(np.tile(b_theta, 4)[:, None])  # [128,1]
    bph4 = np.ascontiguousarray(np.tile(b_phi, 4)[:, None])    # [128,1]
    bg_r = np.ascontiguousarray(b_g[None, :])                  # [1,32]

    xflat = x.reshape(B, C, N)
    in_maps = []
    for core in range(NCORES):
        b, h = divmod(core, 2)
        xrot = np.ascontiguousarray(np.roll(xflat[b], -h * NQ, axis=1))
        xres = np.ascontiguousarray(xrot[:, :NQ] + bo_folded[:, None])
        in_maps.append(
            {
                "xf": xrot,
                "xr": xres,
                "wth": wth4,
                "wph": wph4,
                "wg": wg_r,
                "wo": wo_l,
                "bth": bth4,
                "bph": bph4,
                "bg": bg_r,
            }
        )
    return in_maps


def assemble_out(results):
    out = np.empty((B, C, N), np.float32)
    for core in range(NCORES):
        b, h = divmod(core, 2)
        out[b][:, h * NQ : (h + 1) * NQ] = results[core]["out"]
    return out.reshape(B, C, HH, WW)


_NC_CACHE = [None]


def kernel(**inputs):
    if _NC_CACHE[0] is None:
        _NC_CACHE[0] = build()
    nc = _NC_CACHE[0]
    in_maps = make_in_maps(**inputs)
    res = run_bass_kernel_spmd(nc, in_maps, core_ids=list(range(NCORES)))
    return assemble_out(res.results)



# revision 2
# speedup vs baseline: 1.0043x; 1.0043x over previous
"""NonLocalBlock2D (embedded-gaussian non-local attention) on 8 TRN2 NeuronCores.

v3 — cost-model-driven redesign. Sharding as baseline: core k handles sample
b=k//2, query rows h*3200:(h+1)*3200 (h=k%2); keys are the full 6400 positions
(x rotated per-core so this core's queries are always cols 0:3200).

Cost-model structure (TimelineSim):
  - matmul cost = out_free_size x cycles_per_row (K and ldweights free;
    f32r>=256-wide and bf16 are 1 cyc/row) -> y-stage runs TRANSPOSED:
    e-chunk [128k x 128q] stationary, gT [128k, 33] bf16 moving -> yT
    [128q, 33] costs 33 rows/matmul instead of 512.
  - exp is the 2nd bottleneck: f tiles are built in PAIRS [128, 2, 512] so
    one activation instruction covers 2 chunks (amortizes PSUM access +
    dispatch); pairs split ACT (real Exp, bf16 out) 15 : DVE (one-op
    Schraudolph: round(f*184.66+16250.5) as int16 bits == bf16 exp) 10.
  - ACT runs ONLY Exp (act-table reloads cost 1.3us each).
  - x is DMAed via the Pool queue (25ns dispatch vs 565 on SP) in 16 chunks.
  - gT matmuls are folded into query-block 0's pair loop (lookahead batches)
    to keep the startup critical path short.
  - GPSIMD cannot access PSUM; row-strip tile_position matmuls crash the
    runtime -> output conv uses host-prepared strip-masked weights (wom) so
    every matmul is a full-K base-0 contraction.
  - PSUM: fps ring 3 x [128,2,512] (6 banks) + py ring 2 x [128,512] = 8;
    z conv targets come from the fps ring.
"""

import numpy as np

import concourse.bass as bass
import concourse.tile as tile
from concourse import bacc
from concourse import mybir
from concourse.bass_utils import run_bass_kernel_spmd

B, C, HH, WW = 4, 64, 80, 80
N = HH * WW            # 6400 keys per sample
NQ = N // 2            # 3200 queries per core
INTER = 32
NCORES = 8
MC = 128               # keys per chunk
NMC = N // MC          # 50
NPAIR = NMC // 2       # 25 f/exp pairs per query block
GT_W = INTER + 1       # 32 g-channels + ones column (denominator)

F32 = mybir.dt.float32
F32R = mybir.dt.float32r
BF16 = mybir.dt.bfloat16
I16 = mybir.dt.int16
EXP = mybir.ActivationFunctionType.Exp
ADD = mybir.AluOpType.add
MULT = mybir.AluOpType.mult

BN_EPS = 1e-4

# Schraudolph fast-exp constants (bf16 bit pattern as int16)
A_EXP = 184.6649652337873   # 2^7 * log2(e)
B_EXP = 16250.5             # 2^7 * (127 - 0.0430)

QBLOCKS = [(0, 512), (512, 512), (1024, 512), (1536, 512), (2048, 512),
           (2560, 320), (2880, 320)]

# exp engine per pair, 14 ACT : 11 DVE per 25-pair block (True = ACT)
_acc = 0
EXP_PAT = []
for _p in range(25):
    _acc += 11
    if _acc >= 25:
        _acc -= 25
        EXP_PAT.append(False)
    else:
        EXP_PAT.append(True)
KCON = INTER + 1       # f-matmul contraction: 32 channels + bias/ones row

XSL = [(i * 512, 512) for i in range(12)] + [(6144, 256)]  # x DMA chunks


def _subs(w):
    # 128-query sub-blocks; the last overlaps its predecessor when w is not
    # a multiple of 128 so every py slot has all 128 partitions written
    # (overlapping queries are computed twice, identical values)
    subs = []
    off = 0
    while off + 128 <= w:
        subs.append(off)
        off += 128
    if off < w:
        subs.append(w - 128)
    return subs


def _emit(tc, d):
    nc = tc.nc

    with tc.tile_pool(name="singles", bufs=1) as singles:
        # ---- DMAs: one HWDGE queue serializes launches at ~625ns each, so
        # interleave x chunks with the early-needed params, late stuff last
        xfl = singles.tile([C, N], F32, tag="xfl")
        wmf = singles.tile([C, C], F32, tag="wmf")
        wgf = singles.tile([C, INTER], F32, tag="wgf")
        btau = singles.tile([C, 1], F32, tag="btau")
        bg = singles.tile([128, INTER], F32, tag="bg")
        womf = singles.tile([128, 4 * C], F32, tag="womf")
        xrl = singles.tile([C, NQ], F32, tag="xr")

        def xdma(i):
            off, xw = XSL[i]
            nc.sync.dma_start(xfl[:, off : off + xw], d["xf"][:, off : off + xw])

        xdma(0)
        nc.sync.dma_start(wmf[:], d["wm"][:])
        xdma(1)
        nc.sync.dma_start(wgf[:], d["wg"][:])
        nc.sync.dma_start(btau[:], d["btau"][:])
        xdma(2)
        nc.sync.dma_start(bg[:], d["bg"][0:1, :].partition_broadcast(128))
        for i in range(3, 13):
            xdma(i)
        nc.sync.dma_start(womf[:], d["wom"][:])
        for i in range(4):
            sl = slice(i * 800, (i + 1) * 800)
            nc.sync.dma_start(xrl[:, sl], d["xr"][:, sl])

        # fp32r operands must come from a rounding instruction (walrus);
        # split the rounding copies between DVE and Pool
        xfr = singles.tile([C, N], F32R, tag="xfr")
        for i, (off, xw) in enumerate(XSL):
            sl = slice(off, off + xw)
            if i % 2 == 0:
                nc.vector.tensor_copy(xfr[:, sl], xfl[:, sl])
            else:
                nc.gpsimd.tensor_copy(xfr[:, sl], xfl[:, sl])
        wm = singles.tile([C, C], F32R, tag="wm")
        nc.vector.tensor_copy(wm[:], wmf[:])
        wg = singles.tile([C, INTER], F32R, tag="wg")
        nc.vector.tensor_copy(wg[:], wgf[:])
        wom = singles.tile([128, 4, C], BF16, tag="wom")
        nc.vector.tensor_copy(wom[:].rearrange("p a b -> p (a b)"), womf[:])

        # phi is never materialized: f[q,k] = x_q.T (Wth.T Wph) x_k, so the
        # query side carries th'' = M.T x + Wph.T b_theta (M host-precomputed)
        # and the f-matmul contracts x-chunks (already in SBUF) against th''.
        # The per-query bias terms are softmax-row-invariant and dropped.
        th = singles.tile([C, NQ], F32R, tag="th")
        gt = singles.tile([128, NMC, GT_W], BF16, tag="gt")
        ones50 = singles.tile([128, NMC], BF16, tag="ones50")
        nc.vector.memset(ones50[:], 1.0)
        nc.vector.tensor_copy(
            gt[:, :, INTER : INTER + 1],
            ones50[:].rearrange("p (n o) -> p n o", o=1),
        )
        bgr = singles.tile([128, 8, INTER], F32, tag="bgr")
        for i in range(8):
            nc.vector.tensor_copy(bgr[:, i, :], bg[:])

        # ---- attention (theta convs stream through the fps ring) ----
        with tc.tile_pool(name="fps", bufs=3, space="PSUM") as fps, \
             tc.tile_pool(name="yps", bufs=2, space="PSUM") as yps, \
             tc.tile_pool(name="esb", bufs=12) as esb, \
             tc.tile_pool(name="ep", bufs=4) as ep:

            def gt_batch(c0, nb):
                pgt = fps.tile([128, 2, 512], F32, tag="f")
                pg = pgt[:, 0, : 8 * INTER].rearrange("p (a b) -> p a b", b=INTER)
                for i in range(nb):
                    nc.tensor.matmul(
                        pg[:, i, :],
                        lhsT=xfr[:, (c0 + i) * MC : (c0 + i + 1) * MC],
                        rhs=wg[:],
                        start=(i == 0), stop=(i == nb - 1),
                    )
                nc.vector.tensor_tensor(
                    gt[:, c0 : c0 + nb, :INTER], pg[:, :nb, :], bgr[:, :nb, :],
                    op=ADD,
                )

            def th_slice(q0, w, psrc=None):
                if psrc is None:
                    pt = fps.tile([128, 2, 512], F32, tag="f")
                    pp = pt[0:C, 0, :]
                else:
                    pp = psrc
                nc.tensor.matmul(
                    pp[:, :w], lhsT=wm[:], rhs=xfr[:, q0 : q0 + w],
                    start=True, stop=True,
                )
                nc.vector.tensor_scalar_add(th[:, q0 : q0 + w], pp[:, :w], btau[:])

            def make_pys(py, nsub):
                # CoreSim schedules by dataflow: every PSUM read must
                # data-depend on the region-closing matmul -> copy the whole
                # accumulator out in ONE instruction (hoisted to pair 0 of
                # the next block so the py bank frees early), then work off
                # SBUF.
                def cp():
                    pys = ep.tile([128, 4, GT_W], F32, tag="pys")
                    nc.vector.tensor_copy(pys[:, :nsub, :], py[:, :nsub, :])
                    return pys
                return cp

            def make_epilogue(pyf, pys, subs, q0, w, fin=False):
                def epi():
                    nsub = len(subs)
                    r = ep.tile([128, 4], F32, tag="r")
                    scr = ep.tile([128, 4], F32, tag="scr")
                    dview = pys[:, :, INTER : INTER + 1].rearrange(
                        "p a o -> p (a o)")
                    nc.vector.reciprocal_approx_accurate(
                        r[:, :nsub], dview[:, :nsub], scr[:, :nsub]
                    )
                    # py's data is dead after the pys copy: reuse its bank
                    # for the z conv output (WAR dep via the bt chain)
                    z = pyf[0:C, :]
                    ncv = 0
                    cend = 0
                    conv_i0 = []
                    for soff in subs:
                        i0 = max(0, cend - soff) // 32
                        conv_i0.append(i0)
                        ncv += 4 - i0
                        cend = soff + 128
                    cnt = 0
                    for si, soff in enumerate(subs):
                        ynt = ep.tile([128, INTER], BF16, tag="ynt")
                        nc.gpsimd.tensor_scalar(
                            ynt[:], pys[:, si, :INTER], r[:, si : si + 1],
                            None, op0=MULT,
                        )
                        bt = ep.tile([128, INTER], BF16, tag="bt")
                        nc.vector.transpose(bt[:], ynt[:])
                        for i in range(conv_i0[si], 4):
                            bp = 32 * i
                            nc.tensor.matmul(
                                z[:, soff + bp : soff + bp + 32],
                                lhsT=wom[:, i, :],
                                rhs=bt[:],
                                start=(cnt == 0),
                                stop=(cnt == ncv - 1),
                            )
                            cnt += 1
                    o = ep.tile([C, 512], F32, tag="o")
                    if not fin:
                        nc.vector.tensor_tensor(
                            o[:, :w], z[:, :w], xrl[:, q0 : q0 + w], op=ADD
                        )
                        nc.sync.dma_start(d["out"][:, q0 : q0 + w], o[:, :w])
                    else:
                        # last block: drain per 128-query chunk so the DMAs
                        # launch as soon as each chunk's conv completes
                        for co in range(0, w, 128):
                            ce = min(co + 128, w)
                            nc.vector.tensor_tensor(
                                o[:, co:ce], z[:, co:ce], xrl[:, q0 + co : q0 + ce],
                                op=ADD,
                            )
                            nc.sync.dma_start(
                                d["out"][:, q0 + co : q0 + ce], o[:, co:ce]
                            )
                return epi

            # chunk groups: pairs and singles interleaved [P,S,P,S,...,P] so
            # four fps allocations (2xP + 2xS tiles, 6 banks) are in flight,
            # stretching the exp->f WAR cycle to 6 chunks
            GROUPS = []
            _c = 0
            while _c < NMC:
                if _c + 2 <= NMC:
                    GROUPS.append((_c, 2))
                    _c += 2
                else:
                    GROUPS.append((_c, 1))
                    _c += 1
                if _c + 1 <= NMC and len([g for g in GROUPS if g[1] == 1]) < 16:
                    GROUPS.append((_c, 1))
                    _c += 1
            # engine pattern per group index: balance ACT/DVE work
            def mkpat(na, nd):
                pat, acc = [], 0
                for _ in range(na + nd):
                    acc += nd
                    if acc >= na + nd:
                        acc -= na + nd
                        pat.append(False)
                    else:
                        pat.append(True)
                return pat
            PATP = mkpat(9, 8)    # pairs 9 ACT : 8 DVE
            PATS = mkpat(10, 6)   # singles 10 ACT : 6 DVE

            xcnt = 0
            prev_epi = None
            prev_pys = None
            prev_pyf = None
            for qi, (q0, w) in enumerate(QBLOCKS):
                if qi == 0:
                    th_slice(q0, w)
                subs = _subs(w)
                nsub = len(subs)
                pyf = yps.tile([128, 512], F32, tag="py")
                py = pyf[:, : 4 * GT_W].rearrange("p (a b) -> p a b", b=GT_W)
                pending = []

                # one accumulation group per 2KB psum region: start only on
                # the first matmul into the region, stop only on the last.
                # y matmuls run 2 pairs behind exp so PE never waits on a
                # lagging exp engine.
                def flush_y(py=py, subs=subs, pending=pending):
                    e, c0, gn = pending.pop(0)
                    for j in range(gn):
                        c = c0 + j
                        for si, soff in enumerate(subs):
                            nc.tensor.matmul(
                                py[:, si, :],
                                lhsT=e[:, j, soff : soff + 128],
                                rhs=gt[:, c, :],
                                start=(c == 0 and si == 0),
                                stop=(c == NMC - 1 and si == len(subs) - 1),
                            )

                if qi == 0:
                    gt_batch(0, 8)
                for p in range(NPAIR):
                    pf = fps.tile([128, 2, 512], F32, tag="f")
                    for j in range(2):
                        c = 2 * p + j
                        nc.tensor.matmul(
                            pf[:, j, :w],
                            lhsT=xfr[:, c * MC : (c + 1) * MC],
                            rhs=th[:, q0 : q0 + w],
                            start=True, stop=True,
                        )
                    if qi == 0 and p % 4 == 1 and p < 22:
                        k = (p - 1) // 4
                        gt_batch(8 * k + 8, min(8, NMC - (8 * k + 8)))
                    if p == 0 and prev_pys is not None:
                        prev_pys()
                        prev_pys = None
                    if p == 3 and prev_epi is not None:
                        prev_epi()
                        prev_epi = None
                    if p == 20 and qi + 1 < len(QBLOCKS):
                        # theta conv for the next block lands in the dead py
                        # bank of block qi-1 (its z was consumed early in qi)
                        nq0, nw = QBLOCKS[qi + 1]
                        th_slice(nq0, nw,
                                 psrc=(prev_pyf[0:C, :nw] if prev_pyf is not None
                                       else None))
                    lag = 1 if qi == len(QBLOCKS) - 1 else 3
                    if len(pending) > lag:
                        flush_y()
                    e = esb.tile([128, 2, 512], BF16, tag="e")
                    # block 0: DVE also does the gt biases and startup copies,
                    # so give it fewer fast-exp pairs there (17:8 vs 14:11)
                    use_act = (p % 3 != 1) if qi == 0 else EXP_PAT[xcnt % len(EXP_PAT)]
                    if use_act:
                        nc.scalar.activation(e[:, :, :w], pf[:, :, :w], EXP)
                    else:
                        nc.vector.tensor_scalar(
                            e[:, :, :w].bitcast(I16), pf[:, :, :w],
                            A_EXP, B_EXP, op0=MULT, op1=ADD,
                        )
                    xcnt += 1
                    pending.append((e, 2 * p, 2))
                while pending:
                    flush_y()
                prev_pyf = pyf
                def _mk(pyf=pyf, py=py, subs=subs, q0=q0, w=w):
                    cp = make_pys(py, len(subs))
                    holder = {}
                    def pys_now():
                        holder["pys"] = cp()
                    def epi_now():
                        if "pys" not in holder:
                            holder["pys"] = cp()
                        make_epilogue(pyf, holder["pys"], subs, q0, w,
                                      fin=(q0 + w >= NQ))()
                    return pys_now, epi_now
                prev_pys, prev_epi = _mk()
            prev_pys()
            prev_epi()


def build():
    nc = bacc.Bacc("TRN2", target_bir_lowering=False, debug=False)
    d = {}
    d["xf"] = nc.dram_tensor("xf", [C, N], F32, kind="ExternalInput").ap()
    d["xr"] = nc.dram_tensor("xr", [C, NQ], F32, kind="ExternalInput").ap()
    d["wm"] = nc.dram_tensor("wm", [C, C], F32, kind="ExternalInput").ap()
    d["wg"] = nc.dram_tensor("wg", [C, INTER], F32, kind="ExternalInput").ap()
    d["wom"] = nc.dram_tensor("wom", [128, 4 * C], F32, kind="ExternalInput").ap()
    d["btau"] = nc.dram_tensor("btau", [C, 1], F32, kind="ExternalInput").ap()
    d["bg"] = nc.dram_tensor("bg", [1, INTER], F32, kind="ExternalInput").ap()
    d["out"] = nc.dram_tensor("out", [C, NQ], F32, kind="ExternalOutput").ap()
    with tile.TileContext(nc) as tc:
        _emit(tc, d)
    nc.compile()
    return nc


def make_in_maps(x, w_theta, b_theta, w_phi, b_phi, w_g, b_g,
                 w_out, b_out, bn_gamma, bn_beta, bn_mean, bn_var):
    x = np.ascontiguousarray(np.asarray(x, dtype=np.float32))
    w_theta = np.asarray(w_theta, np.float32)
    b_theta = np.asarray(b_theta, np.float32)
    w_phi = np.asarray(w_phi, np.float32)
    b_phi = np.asarray(b_phi, np.float32)
    w_g = np.asarray(w_g, np.float32)
    b_g = np.asarray(b_g, np.float32)
    w_out = np.asarray(w_out, np.float32)
    b_out = np.asarray(b_out, np.float32)
    bn_gamma = np.asarray(bn_gamma, np.float32)
    bn_beta = np.asarray(bn_beta, np.float32)
    bn_mean = np.asarray(bn_mean, np.float32)
    bn_var = np.asarray(bn_var, np.float32)

    inv = bn_gamma / np.sqrt(bn_var + BN_EPS)
    wo_folded = w_out * inv[:, None]                       # [64,32]
    bo_folded = (b_out - bn_mean) * inv + bn_beta          # [64]

    # f[q,k] = (theta_q+bth).(phi_k+bph) = x_q.T M x_k + (Wph.T bth).x_k
    # + per-query terms that are softmax-row-invariant (dropped);
    # M = Wth.T Wph, and the key-side bias term folds into th'' as a
    # per-channel conv bias.
    wm_l = np.ascontiguousarray(w_theta.T @ w_phi)         # [64,64]
    btau_c = np.ascontiguousarray((w_phi.T @ b_theta)[:, None])  # [64,1]
    wg_r = np.ascontiguousarray(w_g.T)                     # [64,32]
    wom = np.zeros((128, 4, C), np.float32)
    for i in range(4):
        wom[32 * i : 32 * i + 32, i, :] = wo_folded.T
    wom = np.ascontiguousarray(wom.reshape(128, 4 * C))
    bg_r = np.ascontiguousarray(b_g[None, :])              # [1,32]

    xflat = x.reshape(B, C, N)
    in_maps = []
    for core in range(NCORES):
        b, h = divmod(core, 2)
        xrot = np.ascontiguousarray(np.roll(xflat[b], -h * NQ, axis=1))
        xres = np.ascontiguousarray(xrot[:, :NQ] + bo_folded[:, None])
        in_maps.append(
            {
                "xf": xrot,
                "xr": xres,
                "wm": wm_l,
                "wg": wg_r,
                "wom": wom,
                "btau": btau_c,
                "bg": bg_r,
            }
        )
    return in_maps


def assemble_out(results):
    out = np.empty((B, C, N), np.float32)
    for core in range(NCORES):
        b, h = divmod(core, 2)
        out[b][:, h * NQ : (h + 1) * NQ] = results[core]["out"]
    return out.reshape(B, C, HH, WW)


_NC_CACHE = [None]


def kernel(**inputs):
    if _NC_CACHE[0] is None:
        _NC_CACHE[0] = build()
    nc = _NC_CACHE[0]
    in_maps = make_in_maps(**inputs)
    res = run_bass_kernel_spmd(nc, in_maps, core_ids=list(range(NCORES)))
    return assemble_out(res.results)
